# revision 1
# baseline (speedup 1.0000x reference)
"""DSAttention Trainium2 kernel.

Reference computation (per batch b, head h):
    S[q,s]  = (Q[q]·K[s]) * tau[b] + delta[b,s]
    S      += causal mask (s > q -> -inf)
    A       = softmax(S / sqrt(E), axis=s)
    O[q,:]  = sum_s A[q,s] * V[s,:]

Shapes: B=2, L=2048, H=16, E=64 -> 32 (b,h) pairs, 4 per NeuronCore x 8 cores.

Key algebraic folds (host-side prep):
  - tau folds into K:  kt = (K * tau)^T            (bf16, [64, L])
  - delta folds into V: A = exp((QK*tau)/8) * D_s with D_s = exp(delta_s/8);
    O = (V*D)^T A / (D^T A), so v1 = [V*D | D] ([128, NCH, 65] chunks).
  Scores in PSUM are then pure dot products and exp needs only an immediate
  scale -> no per-partition bias anywhere.

Device strategy (per core, per head):
  - Q^T, K^T resident in SBUF as [e=64, L] bf16.
  - Loop over s-chunks n (128 keys): S^T[s,q] = K_n^T.T @ Q^T for q >= 128n
    (causal skip) into a [128,1024] PSUM tile, double buffered.
  - exp() is SPLIT between two engines running in parallel:
      * VectorE takes the first x columns of each chunk (incl. the diagonal
        128-block): one scalar_tensor_tensor computes
        round(ps*c1 + (bias - BIG*tri)) -> int16 with saturation. This is the
        Schraudolph 2^y bit trick emitted directly as bf16 bits; masked
        (s>q) elements saturate to -32768 = 0x8000 = bf16 -0.0.
      * ScalarE takes the remaining columns with an exact exp activation.
  - AV: O^T[65, q] += V1_n.T @ A^T accumulated into [65, 2048-window] PSUM;
    denominator rides in row 64 via the D column of v1.
  - Finalize per 1024-q window: ScalarE copy PSUM->SBUF, PE-transpose each
    128-q chunk, reciprocal of row 64, multiply, DMA out.  Output is O^T
    [64, L] per head; host transposes back.
"""

import math
import sys

sys.path.insert(0, "/opt/trn_rl_repo")

import ml_dtypes
import numpy as np

import concourse.bass as bass
import concourse.tile as tile
from concourse import bacc, mybir
from concourse.masks import make_identity, make_upper_triangular

B, L, H, E = 2, 2048, 16, 64
NCORES = 8
HPC = (B * H) // NCORES  # heads per core = 4
NCH = L // 128  # s-chunks per head = 16
SCALE = 1.0 / 8.0  # 1/sqrt(E)
F32 = mybir.dt.float32
I16 = mybir.dt.int16
BF16 = mybir.dt.bfloat16
EXP = mybir.ActivationFunctionType.Exp
MULT = mybir.AluOpType.mult
ADD = mybir.AluOpType.add

# Schraudolph constants (bf16 bit trick): bits = round(ps*C1 + BIAS)
LOG2E = 1.4426950408889634
C1 = 128.0 * LOG2E * SCALE  # 16*log2(e)
C_ADJ = 7.0  # error-centering constant
BIAS = 127.0 * 128.0 - C_ADJ
BIG = 1.0e9  # pushes masked entries to int16 saturation = bf16 -0.0

# DVE takes the first DVE_DIAG cols of diagonal chunks (>=128 to cover the
# causal mask block) and the first DVE_INT cols of interior chunks.
DVE_DIAG = 288
DVE_INT = 192


def _body(tc, qT, kT, v1, out):
    nc = tc.nc
    from contextlib import ExitStack

    with ExitStack() as ctx:
        const = ctx.enter_context(tc.tile_pool(name="const", bufs=1))
        qk_pool = ctx.enter_context(tc.tile_pool(name="qk", bufs=2))
        v_pool = ctx.enter_context(tc.tile_pool(name="v", bufs=2))
        a_pool = ctx.enter_context(tc.tile_pool(name="a", bufs=5))
        o_pool = ctx.enter_context(tc.tile_pool(name="o", bufs=2))
        r_pool = ctx.enter_context(tc.tile_pool(name="r", bufs=2))
        ps_pool = ctx.enter_context(tc.tile_pool(name="psS", bufs=3, space="PSUM"))
        po_pool = ctx.enter_context(tc.tile_pool(name="psO", bufs=1, space="PSUM"))

        # tribias[:, 0:128]   = BIAS - BIG*[s>q]  (fused causal mask + bias)
        # (diag DVE op uses in1 = tribias[:, 0:x]; cols >=128 = plain BIAS)
        tribias = const.tile([128, DVE_DIAG], F32, name="tribias")
        make_upper_triangular(nc, tribias[:, 0:128], val=BIG, diag=True)
        nc.vector.tensor_scalar(
            tribias[:, 0:128], tribias[:, 0:128], 1.0, BIAS - BIG, MULT, ADD
        )
        nc.vector.memset(tribias[:, 128:DVE_DIAG], BIAS)
        # HAM warm-up: ~12 dense back-to-back matmuls on junk data, emitted
        # before any input-dependent work so they overlap the initial DMA.
        # ~3.4us of sustained PE activity flips the clock gate to 2.4 GHz.
        warm = const.tile([128, 640], BF16, name="warm")
        nc.vector.memset(warm[:], 0.0)
        wps = ps_pool.tile([128, 1024], F32, tag="ps", name="warmps")
        for _ in range(12):
            nc.tensor.matmul(
                wps[:, 0:512], lhsT=warm[:, 0:128], rhs=warm[:, 128:640],
                start=True, stop=True,
            )

        for i in range(HPC):
            qt = qk_pool.tile([128, L], BF16, tag="qt", name=f"qt{i}")
            kt = qk_pool.tile([128, L], BF16, tag="kt", name=f"kt{i}")
            vt = v_pool.tile([128, NCH * 65], BF16, tag="vt", name=f"vt{i}")
            for hf in range(2):
                cs = slice(1024 * hf, 1024 * hf + 1024)
                nc.sync.dma_start(kt[:, cs], kT[i][:, cs])
                nc.sync.dma_start(qt[:, cs], qT[i][:, cs])
                vs = slice(8 * 65 * hf, 8 * 65 * hf + 8 * 65)
                nc.sync.dma_start(vt[:, vs], v1[i][:, vs])

            # Two q-window phases per head: oT is [65, 1024] (2 PSUM
            # banks).  AV matmuls are emitted two units behind their
            # QK/exp so the in-order PE queue never stalls on exp.
            for phase in range(2):
                qlo = 1024 * phase
                qhi = qlo + 1024
                oT = po_pool.tile([65, 1024], F32, tag="oT", name=f"oT{i}_{phase}")
                pend = []  # emitted QK/exp awaiting AV emission
                nlist = list(range(qhi // 128))

                def emit_av(u):
                    n, pieces, a_sb = u
                    for c0, w in pieces:
                        b = (c0 - qlo) // 512
                        j = 2 * phase + b
                        nc.tensor.matmul(
                            oT[:, c0 - qlo : c0 - qlo + w],
                            lhsT=vt[:, n * 65 : n * 65 + 65],
                            rhs=a_sb[:, c0 - qlo : c0 - qlo + w],
                            start=(n == 0),
                            stop=(n == 4 * j + 3),
                        )

                for n in nlist:
                    q0 = max(128 * n, qlo)
                    w = qhi - q0
                    diag = 128 * n >= qlo
                    pieces = []
                    c = q0
                    w0 = 512 * (q0 // 512 + 1) - q0
                    pieces.append((c, w0))
                    c += w0
                    while c < qhi:
                        pieces.append((c, 512))
                        c += 512
                    ps = ps_pool.tile([128, 1024], F32, tag="ps", name=f"ps{i}_{phase}_{n}")
                    for c0, pw in pieces:
                        nc.tensor.matmul(
                            ps[:, c0 - qlo : c0 - qlo + pw],
                            lhsT=kt[:, 128 * n : 128 * n + 128],
                            rhs=qt[:, c0 : c0 + pw],
                            start=True,
                            stop=True,
                        )
                    a_sb = a_pool.tile([128, 1024], BF16, tag="a", name=f"a{i}_{phase}_{n}")
                    r0 = q0 - qlo
                    # exp split: DVE Schraudolph on [r0, r0+x), ACT on the rest
                    x = min(w, DVE_DIAG if diag else DVE_INT)
                    if diag:
                        nc.vector.scalar_tensor_tensor(
                            out=a_sb[:, r0 : r0 + x].bitcast(I16),
                            in0=ps[:, r0 : r0 + x],
                            scalar=C1,
                            in1=tribias[:, 0:x],
                            op0=MULT,
                            op1=ADD,
                        )
                    else:
                        nc.vector.tensor_scalar(
                            a_sb[:, r0 : r0 + x].bitcast(I16),
                            ps[:, r0 : r0 + x],
                            C1,
                            BIAS,
                            MULT,
                            ADD,
                        )
                    if x < w:
                        nc.scalar.activation(
                            a_sb[:, r0 + x : 1024],
                            ps[:, r0 + x : 1024],
                            EXP,
                            scale=SCALE,
                        )
                    pend.append((n, pieces, a_sb))
                    if len(pend) > 3:
                        emit_av(pend.pop(0))
                for u in pend:
                    emit_av(u)

                # Window finalization: PSUM -> SBUF (ScalarE), PE-transpose
                # each 128-q chunk, reciprocal of the denominator row,
                # scale, store.
                o_sb = o_pool.tile([65, 1024], F32, tag="osb", name=f"osb{i}_{phase}")
                nc.vector.tensor_copy(o_sb[:], oT[:, :])
                nc.sync.dma_start(out[i, phase], o_sb[:])


_CACHED = None


def _build():
    global _CACHED
    if _CACHED is not None:
        return _CACHED
    nc = bacc.Bacc("TRN2", target_bir_lowering=False, debug=False)
    qT = nc.dram_tensor("qT", [HPC, 128, L], BF16, kind="ExternalInput").ap()
    kT = nc.dram_tensor("kT", [HPC, 128, L], BF16, kind="ExternalInput").ap()
    v1 = nc.dram_tensor("v1", [HPC, 128, NCH * 65], BF16, kind="ExternalInput").ap()
    out = nc.dram_tensor("out", [HPC, 2, 65, 1024], F32, kind="ExternalOutput").ap()
    with tile.TileContext(nc) as tc:
        _body(tc, qT, kT, v1, out)
    nc.compile()
    _CACHED = nc
    return nc


def _prep_in_maps(queries, keys, values, tau, delta):
    """Shard + relayout the full inputs into 8 per-core input dicts."""
    queries = np.asarray(queries, dtype=np.float32)
    keys = np.asarray(keys, dtype=np.float32)
    values = np.asarray(values, dtype=np.float32)
    tau = np.asarray(tau, dtype=np.float32)
    delta = np.asarray(delta, dtype=np.float32)

    # host folds: K *= tau (per batch); V gets D = exp(delta/8) folded in,
    # with D itself as the 65th AV column (denominator).
    ktau = keys * tau[:, 0][:, None, None, None]
    D = np.exp(delta / 8.0)  # [B, L]
    vD = values * D[:, :, None, None]

    in_maps = []
    for core in range(NCORES):
        qTs = np.zeros((HPC, 128, L), ml_dtypes.bfloat16)
        kTs = np.zeros((HPC, 128, L), ml_dtypes.bfloat16)
        v1s = np.empty((HPC, 128, NCH * 65), ml_dtypes.bfloat16)
        for slot in range(HPC):
            g = core * HPC + slot
            b, h = divmod(g, H)
            qTs[slot, 0:64] = queries[b, :, h, :].T
            kTs[slot, 0:64] = ktau[b, :, h, :].T
            v = vD[b, :, h, :].reshape(NCH, 128, E).transpose(1, 0, 2)
            dd = D[b].reshape(NCH, 128).T[:, :, None]
            vv = np.concatenate([v, dd], axis=2)
            v1s[slot] = vv.reshape(128, NCH * 65).astype(ml_dtypes.bfloat16)
        in_maps.append({"qT": qTs, "kT": kTs, "v1": v1s})
    return in_maps


def _assemble(results):
    O = np.empty((B, L, H, E), np.float32)
    for core in range(NCORES):
        o = results[core]["out"]  # [HPC, 2, 65, 1024] raw O^T + denom row
    # vectorized: stack all cores
    allo = np.stack([results[c]["out"] for c in range(NCORES)])  # [8,HPC,2,65,1024]
    num = allo[:, :, :, 0:64, :]          # [8,HPC,2,64,1024]
    den = allo[:, :, :, 64:65, :]         # [8,HPC,2,1,1024]
    ot = num / den                        # broadcast divide
    # -> [core, slot, L(2*1024), E] via phase concat then transpose
    ot = ot.transpose(0, 1, 2, 4, 3).reshape(NCORES, HPC, L, E)
    for core in range(NCORES):
        for slot in range(HPC):
            g = core * HPC + slot
            b, h = divmod(g, H)
            O[b, :, h, :] = ot[core, slot]
    return O


def run(inputs, trace=False, **kwargs):
    from concourse import bass_utils

    nc = _build()
    in_maps = _prep_in_maps(**inputs)
    res = bass_utils.run_bass_kernel_spmd(
        nc, in_maps, core_ids=list(range(NCORES)), trace=trace, **kwargs
    )
    return _assemble(res.results), res


def kernel(**inputs):
    return run(inputs, trace=False)[0]



# revision 2
# speedup vs baseline: 1.0251x; 1.0251x over previous
"""DSAttention Trainium2 kernel (v2: row-tiled QK + per-chunk exp engines).

Reference computation (per batch b, head h):
    S[q,s]  = (Q[q]·K[s]) * tau[b] + delta[b,s]
    S      += causal mask (s > q -> -inf)
    A       = softmax(S / sqrt(E), axis=s)
    O[q,:]  = sum_s A[q,s] * V[s,:]

Shapes: B=2, L=2048, H=16, E=64 -> 32 (b,h) pairs, 4 per NeuronCore x 8 cores.

Key algebraic folds (host-side prep):
  - tau folds into K:  kt = (K * tau)^T            (bf16, [64, L])
  - delta folds into V: A = exp((QK*tau)/8) * D_s with D_s = exp(delta_s/8);
    O = (V*D)^T A / (D^T A), so v1 = [V*D | D] ([128, NCH, 65] chunks).

v2 changes over the 111us/96us baseline:
  - QK matmuls are ROW-TILED: contraction is E=64, so Q^T/K^T are duplicated
    on SBUF partitions 0-63 and 64-127 and even/odd s-chunks run as
    concurrent 64x128 PE tiles at tile_position (0,0)/(64,0) -> ~2x QK.
  - exp is assigned per-chunk to ONE engine (DVE Schraudolph whole-chunk or
    ScalarE exact exp) instead of column-splitting every chunk: far fewer
    ACTIVATE instructions (ScalarE per-instr overhead ~270ns).
  - oT PSUM is two [65,512] tiles (1 bank each, pool bufs=2): each half is
    evacuated as soon as its accumulation closes (left half mid-phase), so
    phase/head boundaries fully overlap. Left copy on DVE, right on ScalarE.
  - exp ACT table preloaded via a dummy activation before the warmup MMs.
  - next head's input DMAs issued at current head's phase-1 start.
"""

import math
import sys

sys.path.insert(0, "/opt/trn_rl_repo")

import ml_dtypes
import numpy as np

import concourse.bass as bass
import concourse.tile as tile
from concourse import bacc, mybir
from concourse.masks import make_upper_triangular

B, L, H, E = 2, 2048, 16, 64
NCORES = 8
HPC = (B * H) // NCORES  # heads per core = 4
NCH = L // 128  # s-chunks per head = 16
SCALE = 1.0 / 8.0  # 1/sqrt(E)
F32 = mybir.dt.float32
I16 = mybir.dt.int16
BF16 = mybir.dt.bfloat16
EXP = mybir.ActivationFunctionType.Exp
COPYF = mybir.ActivationFunctionType.Copy
MULT = mybir.AluOpType.mult
ADD = mybir.AluOpType.add

# Schraudolph constants (bf16 bit trick): bits = round(ps*C1 + BIAS)
LOG2E = 1.4426950408889634
C1 = 128.0 * LOG2E * SCALE  # 16*log2(e)
C_ADJ = 7.0  # error-centering constant
BIAS = 127.0 * 128.0 - C_ADJ
BIG = 1.0e9  # pushes masked entries to int16 saturation = bf16 -0.0

WARMUP_MMS = 4
LAG = 4  # chunks of QK/exp emitted ahead of their AV

# Per-chunk exp engine assignment (True -> whole chunk on DVE Schraudolph,
# False -> ScalarE exact exp; diag chunks on ScalarE still get a 128-col
# DVE mask stub).  Tuned for DVE 0.96 / ACT 1.2 cols/ns + instr overheads.
PH0_DVE = frozenset({0, 2, 4, 6, 7})
PH1_INT_DVE = frozenset({0, 4})
PH1_DIAG_DVE = frozenset({9, 11, 13, 14, 15})


def _exp_dve_full(phase, n):
    if phase == 0:
        return n in PH0_DVE
    if n < 8:
        return n in PH1_INT_DVE
    return n in PH1_DIAG_DVE


def _body(tc, qT, kT, v1, out):
    nc = tc.nc
    from contextlib import ExitStack

    with ExitStack() as ctx:
        const = ctx.enter_context(tc.tile_pool(name="const", bufs=1))
        qk_pool = ctx.enter_context(tc.tile_pool(name="qk", bufs=2))
        v_pool = ctx.enter_context(tc.tile_pool(name="v", bufs=2))
        a_pool = ctx.enter_context(tc.tile_pool(name="a", bufs=LAG + 2))
        o_pool = ctx.enter_context(tc.tile_pool(name="o", bufs=8))
        ps_pool = ctx.enter_context(tc.tile_pool(name="psS", bufs=3, space="PSUM"))
        po_pool = ctx.enter_context(tc.tile_pool(name="psO", bufs=2, space="PSUM"))

        # Warmup tile first: DVE memset, then PE chews on it while input DMAs
        # land.  The tiny scalar EXP forces the ACT table load during the
        # preamble instead of right before the first real exp.
        warm = const.tile([128, 640], BF16, name="warm")
        nc.vector.memset(warm[:], 0.0)
        tiny = const.tile([128, 8], BF16, name="tiny")
        nc.gpsimd.memset(tiny[:], 0.0)
        nc.scalar.activation(tiny[:, 4:8], tiny[:, 0:4], EXP, scale=SCALE)
        wps = ps_pool.tile([128, 1024], F32, tag="ps", name="warmps")
        for _ in range(WARMUP_MMS):
            nc.tensor.matmul(
                wps[:, 0:512], lhsT=warm[:, 0:128], rhs=warm[:, 128:640],
                start=True, stop=True,
            )

        # tribias[:, 0:128] = BIAS - BIG*[masked]; cols 128..1024 = plain BIAS
        # (a whole-chunk diag STT uses in1 = tribias[:, 0:w]).
        tribias = const.tile([128, 1024], F32, name="tribias")
        make_upper_triangular(nc, tribias[:, 0:128], val=BIG, diag=True)
        nc.vector.tensor_scalar(
            tribias[:, 0:128], tribias[:, 0:128], 1.0, BIAS - BIG, MULT, ADD
        )
        nc.vector.memset(tribias[:, 128:1024], BIAS)

        def dma_in(i, qt, kt, vt):
            for hf in range(2):
                cs = slice(1024 * hf, 1024 * hf + 1024)
                nc.sync.dma_start(kt[:, cs], kT[i][:, cs])
                nc.sync.dma_start(qt[:, cs], qT[i][:, cs])
                vs = slice(8 * 65 * hf, 8 * 65 * hf + 8 * 65)
                nc.sync.dma_start(vt[:, vs], v1[i][:, vs])

        qts, kts, vts = {}, {}, {}

        def alloc_head(i):
            qts[i] = qk_pool.tile([128, L], BF16, tag="qt", name=f"qt{i}")
            kts[i] = qk_pool.tile([128, L], BF16, tag="kt", name=f"kt{i}")
            vts[i] = v_pool.tile([128, NCH * 65], BF16, tag="vt", name=f"vt{i}")
            dma_in(i, qts[i], kts[i], vts[i])

        alloc_head(0)

        for i in range(HPC):
            qt, kt, vt = qts[i], kts[i], vts[i]

            for phase in range(2):
                qlo = 1024 * phase
                qhi = qlo + 1024
                if phase == 1 and i + 1 < HPC:
                    alloc_head(i + 1)  # prefetch next head's inputs

                oL = po_pool.tile([65, 512], F32, tag="oT", name=f"oL{i}_{phase}")
                oR = po_pool.tile([65, 512], F32, tag="oT", name=f"oR{i}_{phase}")
                stopL = 8 * phase + 3  # last chunk contributing to oL
                stopR = 8 * phase + 7
                pend = []

                def emit_av(u):
                    n, pieces, a_sb = u
                    for c0, w in pieces:
                        b = (c0 - qlo) // 512
                        tgt, stop_n = (oL, stopL) if b == 0 else (oR, stopR)
                        col = c0 - qlo - 512 * b
                        nc.tensor.matmul(
                            tgt[:, col : col + w],
                            lhsT=vt[:, n * 65 : n * 65 + 65],
                            rhs=a_sb[:, c0 - qlo : c0 - qlo + w],
                            start=(n == 0),
                            stop=(n == stop_n),
                        )
                        if n == stop_n:
                            osb = o_pool.tile(
                                [65, 512], F32, tag="osb", name=f"osb{i}_{phase}_{b}"
                            )
                            if b == 0:
                                nc.vector.tensor_copy(osb[:], tgt[:])
                            else:
                                nc.scalar.activation(osb[:], tgt[:], COPYF)
                            nc.sync.dma_start(out[i, phase, b], osb[:])

                nmax = qhi // 128
                for p in range(nmax // 2):
                    group = []
                    for n in (2 * p, 2 * p + 1):
                        q0 = max(128 * n, qlo)
                        w = qhi - q0
                        diag = 128 * n >= qlo
                        pieces = []
                        c = q0
                        w0 = 512 * (q0 // 512 + 1) - q0
                        pieces.append((c, w0))
                        c += w0
                        while c < qhi:
                            pieces.append((c, 512))
                            c += 512
                        ps = ps_pool.tile(
                            [128, 1024], F32, tag="ps", name=f"ps{i}_{phase}_{n}"
                        )
                        half = n & 1
                        pr = slice(64 * half, 64 * half + 64)
                        for c0, pw in pieces:
                            nc.tensor.matmul(
                                ps[:, c0 - qlo : c0 - qlo + pw],
                                lhsT=kt[pr, 128 * n : 128 * n + 128],
                                rhs=qt[pr, c0 : c0 + pw],
                                start=True,
                                stop=True,
                                tile_position=(64 * half, 0),
                            )
                        group.append((n, q0, w, diag, pieces, ps))

                    for n, q0, w, diag, pieces, ps in group:
                        a_sb = a_pool.tile(
                            [128, 1024], BF16, tag="a", name=f"a{i}_{phase}_{n}"
                        )
                        r0 = q0 - qlo
                        dve_full = _exp_dve_full(phase, n)
                        if diag:
                            if dve_full:
                                nc.vector.scalar_tensor_tensor(
                                    out=a_sb[:, r0:1024].bitcast(I16),
                                    in0=ps[:, r0:1024],
                                    scalar=C1,
                                    in1=tribias[:, 0 : 1024 - r0],
                                    op0=MULT,
                                    op1=ADD,
                                )
                            else:
                                nc.vector.scalar_tensor_tensor(
                                    out=a_sb[:, r0 : r0 + 128].bitcast(I16),
                                    in0=ps[:, r0 : r0 + 128],
                                    scalar=C1,
                                    in1=tribias[:, 0:128],
                                    op0=MULT,
                                    op1=ADD,
                                )
                                if w > 128:
                                    nc.scalar.activation(
                                        a_sb[:, r0 + 128 : 1024],
                                        ps[:, r0 + 128 : 1024],
                                        EXP,
                                        scale=SCALE,
                                    )
                        else:
                            if dve_full:
                                nc.vector.tensor_scalar(
                                    a_sb[:, 0:1024].bitcast(I16),
                                    ps[:, 0:1024],
                                    C1,
                                    BIAS,
                                    MULT,
                                    ADD,
                                )
                            else:
                                nc.scalar.activation(
                                    a_sb[:, 0:1024], ps[:, 0:1024], EXP, scale=SCALE
                                )
                        pend.append((n, pieces, a_sb))

                    while len(pend) > LAG:
                        emit_av(pend.pop(0))
                for u in pend:
                    emit_av(u)


_CACHED = None


def _build():
    global _CACHED
    if _CACHED is not None:
        return _CACHED
    nc = bacc.Bacc("TRN2", target_bir_lowering=False, debug=False)
    qT = nc.dram_tensor("qT", [HPC, 128, L], BF16, kind="ExternalInput").ap()
    kT = nc.dram_tensor("kT", [HPC, 128, L], BF16, kind="ExternalInput").ap()
    v1 = nc.dram_tensor("v1", [HPC, 128, NCH * 65], BF16, kind="ExternalInput").ap()
    out = nc.dram_tensor("out", [HPC, 2, 2, 65, 512], F32, kind="ExternalOutput").ap()
    with tile.TileContext(nc) as tc:
        _body(tc, qT, kT, v1, out)
    nc.compile()
    _CACHED = nc
    return nc


def _prep_in_maps(queries, keys, values, tau, delta):
    """Shard + relayout the full inputs into 8 per-core input dicts."""
    queries = np.asarray(queries, dtype=np.float32)
    keys = np.asarray(keys, dtype=np.float32)
    values = np.asarray(values, dtype=np.float32)
    tau = np.asarray(tau, dtype=np.float32)
    delta = np.asarray(delta, dtype=np.float32)

    # host folds: K *= tau (per batch); V gets D = exp(delta/8) folded in,
    # with D itself as the 65th AV column (denominator).
    ktau = keys * tau[:, 0][:, None, None, None]
    D = np.exp(delta / 8.0)  # [B, L]
    vD = values * D[:, :, None, None]

    in_maps = []
    for core in range(NCORES):
        qTs = np.empty((HPC, 128, L), ml_dtypes.bfloat16)
        kTs = np.empty((HPC, 128, L), ml_dtypes.bfloat16)
        v1s = np.empty((HPC, 128, NCH * 65), ml_dtypes.bfloat16)
        for slot in range(HPC):
            g = core * HPC + slot
            b, h = divmod(g, H)
            qtv = queries[b, :, h, :].T.astype(ml_dtypes.bfloat16)
            ktv = ktau[b, :, h, :].T.astype(ml_dtypes.bfloat16)
            # duplicate across both partition halves for PE row tiling
            qTs[slot, 0:64] = qtv
            qTs[slot, 64:128] = qtv
            kTs[slot, 0:64] = ktv
            kTs[slot, 64:128] = ktv
            v = vD[b, :, h, :].reshape(NCH, 128, E).transpose(1, 0, 2)
            dd = D[b].reshape(NCH, 128).T[:, :, None]
            vv = np.concatenate([v, dd], axis=2)
            v1s[slot] = vv.reshape(128, NCH * 65).astype(ml_dtypes.bfloat16)
        in_maps.append({"qT": qTs, "kT": kTs, "v1": v1s})
    return in_maps


def _assemble(results):
    O = np.empty((B, L, H, E), np.float32)
    allo = np.stack([results[c]["out"] for c in range(NCORES)])
    # [8, HPC, 2, 2, 65, 512]
    num = allo[:, :, :, :, 0:64, :]
    den = allo[:, :, :, :, 64:65, :]
    ot = num / den
    # -> [core, slot, phase, half, col, e] -> [core, slot, L, E]
    ot = ot.transpose(0, 1, 2, 3, 5, 4).reshape(NCORES, HPC, L, E)
    for core in range(NCORES):
        for slot in range(HPC):
            g = core * HPC + slot
            b, h = divmod(g, H)
            O[b, :, h, :] = ot[core, slot]
    return O


def run(inputs, trace=False, **kwargs):
    from concourse import bass_utils

    nc = _build()
    in_maps = _prep_in_maps(**inputs)
    res = bass_utils.run_bass_kernel_spmd(
        nc, in_maps, core_ids=list(range(NCORES)), trace=trace, **kwargs
    )
    return _assemble(res.results), res


def kernel(**inputs):
    return run(inputs, trace=False)[0]
